# revision 46
# baseline (speedup 1.0000x reference)
"""Trainium2 kernel for the quantum-circuit AENN problem.

The reference applies a fixed 10-qubit variational circuit (186 params) to
each normalized input row, takes |amp|^2, rescales by norm^2, and applies a
Dense layer.  The circuit is LINEAR in the state, so it is a fixed 1024x1024
complex unitary U, and the normalization cancels exactly:

    norm^2 * |U (x/norm)|^2 = |U x|^2

so:  out = ((X @ Ur^T)^2 + (X @ Ui^T)^2) @ kernel + bias

Host side: build U from the 186 weights (tiny), pack W = [Ur^T | Ui^T] in
fp16, pre-transpose X; afterwards apply the small 1024x10 dense layer +
bias to the device-produced probabilities.  Device side (pure data
parallelism, batch sharded 512 rows/core, no collectives): per amp-block
pair t, Y^T = W-block^T x X^T via TensorE (fp16 in, fp32 accumulate),
probs^T = Yr^2 + Yi^2 (ScalarE squares + VectorE add, fp16 out), DMA out.
PE warm-up matmuls lift the HAM clock gate while input DMAs are in
flight; W slabs 2..7 are flow-controlled behind compute so the critical
prefix (slab0 + X^T) gets full HBM bandwidth.
"""

import os
import numpy as np

NUM_QUBITS = 10
LAYER_DEPTH = 4
DIM = 2 ** NUM_QUBITS            # 1024
BATCH = 4096
NUM_OUTPUT = 10
SIZE_ROT = (LAYER_DEPTH + 1) * NUM_QUBITS * 3   # 150
N_CORES = 8
ROWS = BATCH // N_CORES          # 512 rows per core
KT = DIM // 128                  # 8 k-tiles of 128 along the feature dim
AT = DIM // 128                  # 8 amplitude tile-pairs (Re,Im) of 128

_F16 = np.float16
_CACHE = {}
LAST_RESULTS = None  # BassKernelResults of the most recent run (for test.py)


# ----------------------------------------------------------------------------
# Host: build the circuit unitary U (amp = U @ psi)
# ----------------------------------------------------------------------------
def _build_unitary(qw: np.ndarray) -> np.ndarray:
    qw = np.asarray(qw, dtype=np.float64)
    rotations = qw[:SIZE_ROT].reshape(LAYER_DEPTH + 1, NUM_QUBITS, 3)
    rxx = qw[SIZE_ROT:].reshape(LAYER_DEPTH, NUM_QUBITS - 1)

    # Columns of the identity, qubit axes unpacked: shape (2,)*10 + (DIM,)
    M = np.eye(DIM, dtype=np.complex128).reshape((2,) * NUM_QUBITS + (DIM,))

    def apply_r(M, theta, phi, alpha, j):
        sa = np.sin(alpha)
        nx = sa * np.cos(phi)
        ny = sa * np.sin(phi)
        nz = np.cos(alpha)
        ct = np.cos(theta)
        mist = -1j * np.sin(theta)
        U2 = np.array([
            [ct + mist * nz, mist * (nx - 1j * ny)],
            [mist * (nx + 1j * ny), ct - mist * nz],
        ], dtype=np.complex128)
        M = np.tensordot(U2, M, axes=[[1], [j]])
        return np.moveaxis(M, 0, j)

    for k in range(LAYER_DEPTH):
        for j in range(NUM_QUBITS):
            M = apply_r(M, rotations[k, j, 0], rotations[k, j, 1],
                        rotations[k, j, 2], j)
        for j in range(NUM_QUBITS - 1):
            flipped = np.flip(M, axis=(j, j + 1))
            M = np.cos(rxx[k, j]) * M + (-1j * np.sin(rxx[k, j])) * flipped
    for j in range(NUM_QUBITS):
        M = apply_r(M, rotations[LAYER_DEPTH, j, 0],
                    rotations[LAYER_DEPTH, j, 1],
                    rotations[LAYER_DEPTH, j, 2], j)
    return M.reshape(DIM, DIM)   # U with amp = U @ psi


# ----------------------------------------------------------------------------
# Device graph (built once, cached)
# ----------------------------------------------------------------------------
# PE warm-up matmuls: lift the HAM clock gate AND keep PE busy until the
# first real matmul's inputs land (~13.7us) — any PE idle gap can re-throttle
# the clock gate depending on where the free-running HAM window lands.
N_WARMUP = 11


def _build_graph():
    from concourse import bacc
    import concourse.mybir as mybir
    import concourse.tile as tile
    from concourse.tile_rust import add_dep_helper

    f16 = mybir.dt.float16

    nc = bacc.Bacc("TRN2", target_bir_lowering=False, debug=False,
                   num_devices=N_CORES)

    # X^T in two asymmetric row blocks (fp16): pair 0's first quarter is
    # gated on only 0.75MB (slab0 + 256KB xt) of the DMA prefix
    Q = 128
    xta_d = nc.dram_tensor("xta", [128, KT, Q], f16, kind="ExternalInput")
    xtb_d = nc.dram_tensor("xtb", [128, KT, ROWS - Q], f16,
                           kind="ExternalInput")
    # w[t, p, k*256 + j]: j<128 -> Ur[128t+j, 128k+p], j>=128 -> Ui[...]
    w_d = nc.dram_tensor("w", [AT, 128, KT, 256], f16, kind="ExternalInput")
    # probs^T tiles; host applies the 1024x10 dense layer + bias
    out_d = nc.dram_tensor("out", [AT, 128, ROWS], f16, kind="ExternalOutput")

    with tile.TileContext(nc) as tc:
        with (
            tc.tile_pool(name="xtp", bufs=1) as xtp,
            tc.tile_pool(name="wp", bufs=AT) as wp,
            tc.tile_pool(name="cst", bufs=1) as cst,
            tc.tile_pool(name="sq", bufs=2) as sqp,
            tc.tile_pool(name="pb", bufs=2) as pbp,
            tc.tile_pool(name="psmm", bufs=2, space="PSUM") as psmm,
            tc.tile_pool(name="pswu", bufs=1, space="PSUM") as pswu,
        ):
            # PE warm-up on a zeroed scratch tile: no input deps, so these run
            # during the DMA wait and lift the HAM clock gate (PE 1.2 -> 2.4
            # GHz) right as the first real matmul's inputs land.
            scratch = cst.tile([128, ROWS], f16)
            nc.gpsimd.memset(scratch[:], 0.0)
            wu_ps = pswu.tile([128, ROWS], mybir.dt.float32)
            for _ in range(N_WARMUP):
                nc.tensor.matmul(wu_ps[:], scratch[:, 0:128], scratch[:],
                                 start=True, stop=True, skip_group_check=True)

            # Critical prefix on the sync HWDGE ring in consumption order:
            # slab0, xt, slab1.  Slabs 2-7 on gpsimd SWDGE, gated on pair
            # t-2's compute so they don't steal HBM bandwidth from the prefix.
            w_slabs = [wp.tile([128, KT, 256], f16, name=f"wt{t}", tag="wt")
                       for t in range(AT)]
            w_dmas = [None] * AT
            xt_sb = xtp.tile([128, KT, ROWS], f16)
            # consumption-ordered critical prefix on the sync ring
            w_dmas[0] = nc.sync.dma_start(out=w_slabs[0][:], in_=w_d[0])
            nc.sync.dma_start(out=xt_sb[:, :, 0:Q], in_=xta_d[:])
            nc.sync.dma_start(out=xt_sb[:, :, Q:ROWS], in_=xtb_d[:])
            w_dmas[1] = nc.sync.dma_start(out=w_slabs[1][:], in_=w_d[1])
            for t in range(2, AT):
                w_dmas[t] = nc.gpsimd.dma_start(out=w_slabs[t][:],
                                                in_=w_d[t])

            def half_pair(t, wt, r0, nr):
                """One amp-pair over rows [r0, r0+nr): 16 matmuls + epilogue."""
                ps_re = psmm.tile([128, ROWS], mybir.dt.float32, tag="ps_re")
                ps_im = psmm.tile([128, ROWS], mybir.dt.float32, tag="ps_im")
                for k in range(KT):
                    m = nc.tensor.matmul(ps_re[:, 0:nr], wt[:, k, 0:128],
                                         xt_sb[:, k, r0:r0 + nr],
                                         start=(k == 0), stop=(k == KT - 1))
                    if k == 0 and r0 == 0:
                        # release staggered slab DMAs: slabs 2,3 when pair 0
                        # starts, slab t+3 when pair t starts (one extra pair
                        # of margin over consumption)
                        deps = [2, 3] if t == 0 else (
                            [t + 3] if t + 3 < AT else [])
                        for tt in deps:
                            add_dep_helper(w_dmas[tt].ins, m.ins, sync=True,
                                           reason="stagger W slab DMAs")
                for k in range(KT):
                    nc.tensor.matmul(ps_im[:, 0:nr], wt[:, k, 128:256],
                                     xt_sb[:, k, r0:r0 + nr],
                                     start=(k == 0), stop=(k == KT - 1))
                sq = sqp.tile([128, 2, ROWS], mybir.dt.float32, tag="sq")
                nc.scalar.square(sq[:, 0, 0:nr], ps_re[:, 0:nr])
                nc.scalar.square(sq[:, 1, 0:nr], ps_im[:, 0:nr])
                p_t = pbp.tile([128, ROWS], f16, tag="p_t")
                nc.vector.tensor_add(p_t[:, 0:nr], sq[:, 0, 0:nr],
                                     sq[:, 1, 0:nr])
                nc.sync.dma_start(out=out_d[t][:, r0:r0 + nr],
                                  in_=p_t[:, 0:nr])

            for t in range(AT):
                if 0 < t < AT - 1:
                    half_pair(t, w_slabs[t], 0, ROWS)
                else:
                    # first pair: asymmetric row split so compute starts on
                    # the small first xt block; last pair: row-halves so its
                    # epilogue pipelines with the second half's matmuls
                    # instead of serializing at the very end of the kernel
                    if t == 0:
                        half_pair(t, w_slabs[t], 0, Q)
                        half_pair(t, w_slabs[t], Q, ROWS - Q)
                    else:
                        half_pair(t, w_slabs[t], 0, ROWS // 2)
                        half_pair(t, w_slabs[t], ROWS // 2, ROWS // 2)

    nc.compile()
    return nc


def _ensure_ntff_hook():
    """The trace path does `from antenv.axon_hooks import ...`; some images
    lack that optional module.  Provide it (wired to the axon PJRT .so when
    available) so BASS_TRACE=1 profiles instead of crashing."""
    try:
        import antenv.axon_hooks  # noqa: F401
        return
    except ImportError:
        pass
    import sys
    import types
    try:
        import antenv
    except ImportError:
        return
    mod = types.ModuleType("antenv.axon_hooks")
    state = {"hook": None}
    mod.set_axon_ntff_profile_hook = lambda h: state.__setitem__("hook", h)
    mod.get_axon_ntff_profile_hook = lambda: state["hook"]
    sys.modules["antenv.axon_hooks"] = mod
    antenv.axon_hooks = mod
    try:
        from trn_agent_boot.trn_boot import _ntff_profile_via_ctypes
        so_path = "/opt/axon/libaxon_pjrt.so"
        if os.path.exists(so_path):
            hook = _ntff_profile_via_ctypes(so_path)
            if hook is not None:
                mod.set_axon_ntff_profile_hook(hook)
    except Exception:
        pass


# ----------------------------------------------------------------------------
# Entry point
# ----------------------------------------------------------------------------
def kernel(x, quantum_weights, kernel, bias):
    global LAST_RESULTS
    _ensure_ntff_hook()
    from concourse.bass_utils import run_bass_kernel_spmd

    x = np.asarray(x, dtype=np.float32)
    qw = np.asarray(quantum_weights, dtype=np.float32)
    kmat = np.asarray(kernel, dtype=np.float32)
    bvec = np.asarray(bias, dtype=np.float32)

    U = _build_unitary(qw)
    # w[t, p, k, j]: j<128 -> Ur[128t+j, 128k+p]; j>=128 -> Ui[128t+j-128, 128k+p]
    Ur4 = U.real.reshape(AT, 128, KT, 128).transpose(0, 2, 3, 1)  # [t, k, p, j]
    Ui4 = U.imag.reshape(AT, 128, KT, 128).transpose(0, 2, 3, 1)
    w4 = np.concatenate([Ur4, Ui4], axis=3)                # [AT, KT, 128, 256]
    w4 = np.ascontiguousarray(w4.transpose(0, 2, 1, 3)).astype(_F16)  # [t,p,k,j]

    if "nc" not in _CACHE:
        _CACHE["nc"] = _build_graph()
    nc = _CACHE["nc"]

    in_maps = []
    for c in range(N_CORES):
        xs = x[c * ROWS:(c + 1) * ROWS]                        # [512, 1024]
        xtT = xs.T.reshape(KT, 128, ROWS)
        xta = np.ascontiguousarray(
            xtT[:, :, 0:128].transpose(1, 0, 2)).astype(_F16)
        xtb = np.ascontiguousarray(
            xtT[:, :, 128:ROWS].transpose(1, 0, 2)).astype(_F16)
        in_maps.append({"xta": xta, "xtb": xtb, "w": w4})

    res = run_bass_kernel_spmd(nc, in_maps, core_ids=list(range(N_CORES)))
    LAST_RESULTS = res
    out = np.empty((BATCH, NUM_OUTPUT), dtype=np.float32)
    for c in range(N_CORES):
        # device emits probs^T blocks: out_d[t, j, r] = probs[r, 128t+j]
        probs = res.results[c]["out"].astype(np.float32)
        probs = probs.transpose(2, 0, 1).reshape(ROWS, DIM)
        out[c * ROWS:(c + 1) * ROWS] = probs @ kmat + bvec
    return out


# revision 47
# speedup vs baseline: 1.0105x; 1.0105x over previous
"""Trainium2 kernel for the quantum-circuit AENN problem.

The reference applies a fixed 10-qubit variational circuit (186 params) to
each normalized input row, takes |amp|^2, rescales by norm^2, and applies a
Dense layer.  The circuit is LINEAR in the state, so it is a fixed 1024x1024
complex unitary U, and the normalization cancels exactly:

    norm^2 * |U (x/norm)|^2 = |U x|^2

so:  out = ((X @ Ur^T)^2 + (X @ Ui^T)^2) @ kernel + bias

Host side: build U from the 186 weights (tiny), pack W = [Ur^T | Ui^T] in
fp16, pre-transpose X; afterwards apply the small 1024x10 dense layer +
bias to the device-produced probabilities.  Device side (pure data
parallelism, batch sharded 512 rows/core, no collectives): per amp-block
pair t, Y^T = W-block^T x X^T via TensorE (fp16 in, fp32 accumulate),
probs^T = Yr^2 + Yi^2 (ScalarE squares + VectorE add, fp16 out), DMA out.
PE warm-up matmuls lift the HAM clock gate while input DMAs are in
flight; W slabs 2..7 are flow-controlled behind compute so the critical
prefix (slab0 + X^T) gets full HBM bandwidth.
"""

import os
import numpy as np

NUM_QUBITS = 10
LAYER_DEPTH = 4
DIM = 2 ** NUM_QUBITS            # 1024
BATCH = 4096
NUM_OUTPUT = 10
SIZE_ROT = (LAYER_DEPTH + 1) * NUM_QUBITS * 3   # 150
N_CORES = 8
ROWS = BATCH // N_CORES          # 512 rows per core
KT = DIM // 128                  # 8 k-tiles of 128 along the feature dim
AT = DIM // 128                  # 8 amplitude tile-pairs (Re,Im) of 128

_F16 = np.float16
_CACHE = {}
LAST_RESULTS = None  # BassKernelResults of the most recent run (for test.py)


# ----------------------------------------------------------------------------
# Host: build the circuit unitary U (amp = U @ psi)
# ----------------------------------------------------------------------------
def _build_unitary(qw: np.ndarray) -> np.ndarray:
    qw = np.asarray(qw, dtype=np.float64)
    rotations = qw[:SIZE_ROT].reshape(LAYER_DEPTH + 1, NUM_QUBITS, 3)
    rxx = qw[SIZE_ROT:].reshape(LAYER_DEPTH, NUM_QUBITS - 1)

    # Columns of the identity, qubit axes unpacked: shape (2,)*10 + (DIM,)
    M = np.eye(DIM, dtype=np.complex128).reshape((2,) * NUM_QUBITS + (DIM,))

    def apply_r(M, theta, phi, alpha, j):
        sa = np.sin(alpha)
        nx = sa * np.cos(phi)
        ny = sa * np.sin(phi)
        nz = np.cos(alpha)
        ct = np.cos(theta)
        mist = -1j * np.sin(theta)
        U2 = np.array([
            [ct + mist * nz, mist * (nx - 1j * ny)],
            [mist * (nx + 1j * ny), ct - mist * nz],
        ], dtype=np.complex128)
        M = np.tensordot(U2, M, axes=[[1], [j]])
        return np.moveaxis(M, 0, j)

    for k in range(LAYER_DEPTH):
        for j in range(NUM_QUBITS):
            M = apply_r(M, rotations[k, j, 0], rotations[k, j, 1],
                        rotations[k, j, 2], j)
        for j in range(NUM_QUBITS - 1):
            flipped = np.flip(M, axis=(j, j + 1))
            M = np.cos(rxx[k, j]) * M + (-1j * np.sin(rxx[k, j])) * flipped
    for j in range(NUM_QUBITS):
        M = apply_r(M, rotations[LAYER_DEPTH, j, 0],
                    rotations[LAYER_DEPTH, j, 1],
                    rotations[LAYER_DEPTH, j, 2], j)
    return M.reshape(DIM, DIM)   # U with amp = U @ psi


# ----------------------------------------------------------------------------
# Device graph (built once, cached)
# ----------------------------------------------------------------------------
# PE warm-up matmuls: lift the HAM clock gate AND keep PE busy until the
# first real matmul's inputs land (~13.7us) — any PE idle gap can re-throttle
# the clock gate depending on where the free-running HAM window lands.
N_WARMUP = 13


def _build_graph():
    from concourse import bacc
    import concourse.mybir as mybir
    import concourse.tile as tile
    from concourse.tile_rust import add_dep_helper

    f16 = mybir.dt.float16

    nc = bacc.Bacc("TRN2", target_bir_lowering=False, debug=False,
                   num_devices=N_CORES)

    # xt[h, p, k, r'] = X[256h + r', 128k+p] (fp16) — two row-half DMAs so
    # pair 0's first row-half is gated on only 1.0MB (slab0 + xt half) of
    # the DMA prefix instead of 1.5MB
    xt_d = nc.dram_tensor("xt", [2, 128, KT, ROWS // 2], f16,
                          kind="ExternalInput")
    # w[t, p, k*256 + j]: j<128 -> Ur[128t+j, 128k+p], j>=128 -> Ui[...]
    w_d = nc.dram_tensor("w", [AT, 128, KT, 256], f16, kind="ExternalInput")
    # probs^T tiles; host applies the 1024x10 dense layer + bias
    out_d = nc.dram_tensor("out", [AT, 128, ROWS], f16, kind="ExternalOutput")

    with tile.TileContext(nc) as tc:
        with (
            tc.tile_pool(name="xtp", bufs=1) as xtp,
            tc.tile_pool(name="wp", bufs=AT) as wp,
            tc.tile_pool(name="cst", bufs=1) as cst,
            tc.tile_pool(name="sq", bufs=2) as sqp,
            tc.tile_pool(name="pb", bufs=2) as pbp,
            tc.tile_pool(name="psmm", bufs=2, space="PSUM") as psmm,
            tc.tile_pool(name="pswu", bufs=1, space="PSUM") as pswu,
        ):
            # PE warm-up on a zeroed scratch tile: no input deps, so these run
            # during the DMA wait and lift the HAM clock gate (PE 1.2 -> 2.4
            # GHz) right as the first real matmul's inputs land.
            scratch = cst.tile([128, ROWS], f16)
            nc.gpsimd.memset(scratch[:], 0.0)
            wu_ps = pswu.tile([128, ROWS], mybir.dt.float32)
            for _ in range(N_WARMUP):
                nc.tensor.matmul(wu_ps[:], scratch[:, 0:128], scratch[:],
                                 start=True, stop=True, skip_group_check=True)

            # Critical prefix on the sync HWDGE ring in consumption order:
            # slab0, xt, slab1.  Slabs 2-7 on gpsimd SWDGE, gated on pair
            # t-2's compute so they don't steal HBM bandwidth from the prefix.
            w_slabs = [wp.tile([128, KT, 256], f16, name=f"wt{t}", tag="wt")
                       for t in range(AT)]
            w_dmas = [None] * AT
            xt_sb = xtp.tile([128, KT, ROWS], f16)
            H = ROWS // 2
            # consumption-ordered critical prefix on the sync ring
            w_dmas[0] = nc.sync.dma_start(out=w_slabs[0][:], in_=w_d[0])
            nc.sync.dma_start(out=xt_sb[:, :, 0:H], in_=xt_d[0])
            nc.sync.dma_start(out=xt_sb[:, :, H:ROWS], in_=xt_d[1])
            w_dmas[1] = nc.sync.dma_start(out=w_slabs[1][:], in_=w_d[1])
            for t in range(2, AT):
                w_dmas[t] = nc.gpsimd.dma_start(out=w_slabs[t][:],
                                                in_=w_d[t])

            def half_pair(t, wt, r0, nr):
                """One amp-pair over rows [r0, r0+nr): 16 matmuls + epilogue."""
                ps_re = psmm.tile([128, ROWS], mybir.dt.float32, tag="ps_re")
                ps_im = psmm.tile([128, ROWS], mybir.dt.float32, tag="ps_im")
                for k in range(KT):
                    m = nc.tensor.matmul(ps_re[:, 0:nr], wt[:, k, 0:128],
                                         xt_sb[:, k, r0:r0 + nr],
                                         start=(k == 0), stop=(k == KT - 1))
                    if k == 0 and r0 == 0:
                        # release staggered slab DMAs: slabs 2,3 when pair 0
                        # starts, slab t+3 when pair t starts (one extra pair
                        # of margin over consumption)
                        deps = [2, 3] if t == 0 else (
                            [t + 3] if t + 3 < AT else [])
                        for tt in deps:
                            add_dep_helper(w_dmas[tt].ins, m.ins, sync=True,
                                           reason="stagger W slab DMAs")
                for k in range(KT):
                    nc.tensor.matmul(ps_im[:, 0:nr], wt[:, k, 128:256],
                                     xt_sb[:, k, r0:r0 + nr],
                                     start=(k == 0), stop=(k == KT - 1))
                sq = sqp.tile([128, 2, ROWS], mybir.dt.float32, tag="sq")
                nc.scalar.square(sq[:, 0, 0:nr], ps_re[:, 0:nr])
                nc.scalar.square(sq[:, 1, 0:nr], ps_im[:, 0:nr])
                p_t = pbp.tile([128, ROWS], f16, tag="p_t")
                nc.vector.tensor_add(p_t[:, 0:nr], sq[:, 0, 0:nr],
                                     sq[:, 1, 0:nr])
                nc.sync.dma_start(out=out_d[t][:, r0:r0 + nr],
                                  in_=p_t[:, 0:nr])

            for t in range(AT):
                if 0 < t < AT - 1:
                    half_pair(t, w_slabs[t], 0, ROWS)
                else:
                    # first pair: row-halves so compute starts on the first
                    # xt half-DMA; last pair: row-halves so its epilogue
                    # pipelines with the second half's matmuls instead of
                    # serializing at the very end of the kernel
                    half_pair(t, w_slabs[t], 0, ROWS // 2)
                    half_pair(t, w_slabs[t], ROWS // 2, ROWS // 2)

    nc.compile()
    return nc


def _ensure_ntff_hook():
    """The trace path does `from antenv.axon_hooks import ...`; some images
    lack that optional module.  Provide it (wired to the axon PJRT .so when
    available) so BASS_TRACE=1 profiles instead of crashing."""
    try:
        import antenv.axon_hooks  # noqa: F401
        return
    except ImportError:
        pass
    import sys
    import types
    try:
        import antenv
    except ImportError:
        return
    mod = types.ModuleType("antenv.axon_hooks")
    state = {"hook": None}
    mod.set_axon_ntff_profile_hook = lambda h: state.__setitem__("hook", h)
    mod.get_axon_ntff_profile_hook = lambda: state["hook"]
    sys.modules["antenv.axon_hooks"] = mod
    antenv.axon_hooks = mod
    try:
        from trn_agent_boot.trn_boot import _ntff_profile_via_ctypes
        so_path = "/opt/axon/libaxon_pjrt.so"
        if os.path.exists(so_path):
            hook = _ntff_profile_via_ctypes(so_path)
            if hook is not None:
                mod.set_axon_ntff_profile_hook(hook)
    except Exception:
        pass


# ----------------------------------------------------------------------------
# Entry point
# ----------------------------------------------------------------------------
def kernel(x, quantum_weights, kernel, bias):
    global LAST_RESULTS
    _ensure_ntff_hook()
    from concourse.bass_utils import run_bass_kernel_spmd

    x = np.asarray(x, dtype=np.float32)
    qw = np.asarray(quantum_weights, dtype=np.float32)
    kmat = np.asarray(kernel, dtype=np.float32)
    bvec = np.asarray(bias, dtype=np.float32)

    U = _build_unitary(qw)
    # w[t, p, k, j]: j<128 -> Ur[128t+j, 128k+p]; j>=128 -> Ui[128t+j-128, 128k+p]
    Ur4 = U.real.reshape(AT, 128, KT, 128).transpose(0, 2, 3, 1)  # [t, k, p, j]
    Ui4 = U.imag.reshape(AT, 128, KT, 128).transpose(0, 2, 3, 1)
    w4 = np.concatenate([Ur4, Ui4], axis=3)                # [AT, KT, 128, 256]
    w4 = np.ascontiguousarray(w4.transpose(0, 2, 1, 3)).astype(_F16)  # [t,p,k,j]

    if "nc" not in _CACHE:
        _CACHE["nc"] = _build_graph()
    nc = _CACHE["nc"]

    in_maps = []
    for c in range(N_CORES):
        xs = x[c * ROWS:(c + 1) * ROWS]                        # [512, 1024]
        # xt[h, p, k, r'] = X[256h + r', 128k+p]
        xt = np.ascontiguousarray(
            xs.T.reshape(KT, 128, 2, ROWS // 2).transpose(2, 1, 0, 3)
        ).astype(_F16)
        in_maps.append({"xt": xt, "w": w4})

    res = run_bass_kernel_spmd(nc, in_maps, core_ids=list(range(N_CORES)))
    LAST_RESULTS = res
    out = np.empty((BATCH, NUM_OUTPUT), dtype=np.float32)
    for c in range(N_CORES):
        # device emits probs^T blocks: out_d[t, j, r] = probs[r, 128t+j]
        probs = res.results[c]["out"].astype(np.float32)
        probs = probs.transpose(2, 0, 1).reshape(ROWS, DIM)
        out[c * ROWS:(c + 1) * ROWS] = probs @ kmat + bvec
    return out
